# revision 27
# baseline (speedup 1.0000x reference)
"""Causal multi-head attention (B=4, S=2048, D=1024, H=16, hd=64) on 8
Trainium2 NeuronCores.

Sharding: batch (4-way) x head-group (2-way). Core c handles batch c//2 and
heads [8*(c%2), 8*(c%2)+8). Each core computes its heads' contribution to the
output projection; the host sums the two partials per batch and adds bo.

v3: bf16 datapath, restructured schedule:
  - x^T resident in SBUF; phase A computes K, Q AND V projections per
    256-col half-window (K/Q psum tiles in pool A, V in pool B), so the
    attention phase is pure S/exp/V/norm/O work.
  - PSUM: pool A (2x[128,1024]) holds attention o2 accumulators (and
    phase-A K/Q), pool B (2x[128,1024]) holds score tiles + out-proj
    tiles (and phase-A V) - accumulators never block the score ring.
  - causal mask added (-1e30) on PSUM scores pre-exp (DVE).
  - V stationary carries 64 ones-columns so attn@V replicates the softmax
    denominator z across PSUM rows 64-127; norm = GpSimd copy to SBUF +
    per-parity reciprocal_approx_fast + DVE muls.
  - out-projection of window w deferred past the first score batch of
    window w+1; output DMA'd directly from PSUM.
"""
import numpy as np
import ml_dtypes

import concourse.mybir as mybir
from concourse import bacc
from concourse.tile import TileContext
from concourse.bass_utils import run_bass_kernel_spmd

FP32 = mybir.dt.float32
BF16 = mybir.dt.bfloat16
EXP = mybir.ActivationFunctionType.Exp
BF16NP = ml_dtypes.bfloat16

B, S, D = 4, 2048, 1024
H, HD = 16, 64
NCORES = 8
HPG = 8              # heads per group (per core)
GD = HPG * HD        # 512: group head-dim width
W = 512              # query window
NW = S // W          # 4
HW = 256             # phase-A half-window
NHW = S // HW        # 8
KCH = 128            # key chunk
NKC = S // KCH       # 16
DC = 128             # D contraction chunk
NDC = D // DC        # 8
SCALE = 1.0 / 8.0    # 1/sqrt(hd)
NEG = -1.0e30

_CACHE = {}


def _build_program():
    nc = bacc.Bacc("TRN2", target_bir_lowering=False, debug=False,
                   num_devices=NCORES)

    xT = nc.dram_tensor("xT", [D, S], BF16, kind="ExternalInput").ap()
    wq = nc.dram_tensor("wq", [D, GD], BF16, kind="ExternalInput").ap()
    wk = nc.dram_tensor("wk", [D, GD], BF16, kind="ExternalInput").ap()
    wv = nc.dram_tensor("wv", [D, GD], BF16, kind="ExternalInput").ap()
    wo = nc.dram_tensor("wo", [GD, D], BF16, kind="ExternalInput").ap()
    bq2 = nc.dram_tensor("bq2", [128, 4], FP32, kind="ExternalInput").ap()
    bk2 = nc.dram_tensor("bk2", [128, 4], FP32, kind="ExternalInput").ap()
    out = nc.dram_tensor("out", [S, D], FP32, kind="ExternalOutput").ap()

    with TileContext(nc) as tc:
        with (
            tc.tile_pool(name="xta", bufs=8) as xta_pool,
            tc.tile_pool(name="xtb", bufs=8) as xtb_pool,
            tc.tile_pool(name="wts", bufs=32) as wts_pool,
            tc.tile_pool(name="kt", bufs=4) as kt_pool,
            tc.tile_pool(name="vst", bufs=16) as v_pool,
            tc.tile_pool(name="qt", bufs=16) as qt_pool,
            tc.tile_pool(name="et", bufs=9) as exp_pool,
            tc.tile_pool(name="ao", bufs=6) as ao_pool,
            tc.tile_pool(name="zz", bufs=4) as zz_pool,
            tc.tile_pool(name="cst", bufs=1) as cst_pool,
            tc.tile_pool(name="ob", bufs=3) as out_pool,
            tc.tile_pool(name="psO", bufs=1, space="PSUM") as psO,
            tc.tile_pool(name="psP", bufs=2, space="PSUM") as psP,
            tc.tile_pool(name="psB", bufs=2, space="PSUM") as psB,
        ):
            # ---- constants: biases, additive causal mask ----
            bq_t = cst_pool.tile([128, 4], FP32, tag="bq")
            bk_t = cst_pool.tile([128, 4], FP32, tag="bk")
            # tri[p, j] (both 128-col halves): 0 where col>=partition
            # (causal-valid), -1e30 where col<partition.
            tri = cst_pool.tile([128, 256], FP32, tag="tri")
            nc.gpsimd.memset(tri[:], 0.0)
            for half in range(2):
                nc.gpsimd.affine_select(
                    out=tri[:, half * 128:(half + 1) * 128],
                    in_=tri[:, half * 128:(half + 1) * 128],
                    compare_op=mybir.AluOpType.is_ge,
                    fill=NEG, base=0, pattern=[[1, 128]],
                    channel_multiplier=-1,
                )

            # ---- resident x^T: window-0 slice first so phase A can start
            # after ~3MB instead of ~5MB of DMA ----
            xtA = [xta_pool.tile([128, W], BF16, tag="xta", name=f"xta{i}")
                   for i in range(NDC)]
            xtB = [xtb_pool.tile([128, S - W], BF16, tag="xtb", name=f"xtb{i}")
                   for i in range(NDC)]
            for dc in range(NDC):
                nc.gpsimd.dma_start(out=xtA[dc][:],
                                    in_=xT[dc * DC:(dc + 1) * DC, 0:W])
            for dc in range(NDC):
                nc.gpsimd.dma_start(out=xtB[dc][:],
                                    in_=xT[dc * DC:(dc + 1) * DC, W:S])

            def xt_slice(dc, s0, n):
                if s0 + n <= W:
                    return xtA[dc][:, s0:s0 + n]
                return xtB[dc][:, s0 - W:s0 - W + n]

            wk_tiles = [wts_pool.tile([128, GD], BF16, tag="w", name=f"wkt{i}")
                        for i in range(NDC)]
            for dc in range(NDC):
                nc.scalar.dma_start(out=wk_tiles[dc][:],
                                    in_=wk[dc * DC:(dc + 1) * DC, :])
            nc.scalar.dma_start(out=bq_t[:], in_=bq2[:])
            nc.scalar.dma_start(out=bk_t[:], in_=bk2[:])

            # ---- persistent SBUF tensors ----
            kt_tiles = [kt_pool.tile([128, S], BF16, tag="kt", name=f"kt{i}")
                        for i in range(4)]
            v_tiles = [v_pool.tile([128, 8 * 128], BF16, tag="v", name=f"v{i}")
                       for i in range(NKC)]
            for kc in range(NKC):
                ones_ap = v_tiles[kc][:].rearrange(
                    "p (h e) -> p h e", e=128)[:, :, 64:128]
                nc.gpsimd.memset(ones_ap, 1.0)
            # qt_all[w][hp]: [128, 512] bf16 per window per head-pair
            qt_all = [[qt_pool.tile([128, W], BF16, tag="qt",
                                    name=f"qt{w}_{hp}") for hp in range(4)]
                      for w in range(NW)]

            # ---- remaining weights: wq (scalar q), wv/wo (sync q) ----
            wq_tiles = [wts_pool.tile([128, GD], BF16, tag="w", name=f"wqt{i}")
                        for i in range(NDC)]
            for dc in range(NDC):
                nc.scalar.dma_start(out=wq_tiles[dc][:],
                                    in_=wq[dc * DC:(dc + 1) * DC, :])
            wv_tiles = [wts_pool.tile([128, GD], BF16, tag="w", name=f"wvt{i}")
                        for i in range(NDC)]
            for dc in range(NDC):
                nc.sync.dma_start(out=wv_tiles[dc][:],
                                  in_=wv[dc * DC:(dc + 1) * DC, :])
            wo_tiles = {}
            for hc in range(4):
                for dcol in range(2):
                    t = wts_pool.tile([128, 512], BF16, tag="w",
                                      name=f"wot{hc}_{dcol}")
                    nc.sync.dma_start(
                        out=t[:], in_=wo[hc * 128:(hc + 1) * 128,
                                         dcol * 512:(dcol + 1) * 512])
                    wo_tiles[(hc, dcol)] = t

            # ---- projection passes: each = one psP tile, 16 matmuls ----
            # K pass g: heads-pairs (2g, 2g+1); V pass g: seq chunks
            # (2g, 2g+1); Q pass g: like K. Evictions: ACT (Identity+bias)
            # for window 0 (pre-attention, ACT idle), DVE when interleaved.
            def pass_K(w, g, on_act):
                s0 = w * W
                ps2 = [psP.tile([128, 512], FP32, tag="pp",
                                name=f"kp{w}_{g}_{i}") for i in range(2)]
                for dc in range(NDC):
                    xt = xt_slice(dc, s0, W)
                    st, sp = (dc == 0), (dc == NDC - 1)
                    for i in range(2):
                        hp = 2 * g + i
                        nc.tensor.matmul(
                            ps2[i][:],
                            wk_tiles[dc][:, hp * 128:(hp + 1) * 128],
                            xt, start=st, stop=sp)
                for i in range(2):
                    hp = 2 * g + i
                    if on_act:
                        nc.scalar.activation(
                            kt_tiles[hp][:, s0:s0 + W], ps2[i][:],
                            mybir.ActivationFunctionType.Identity,
                            bias=bk_t[:, hp:hp + 1], scale=1.0)
                    else:
                        nc.vector.tensor_scalar_add(
                            kt_tiles[hp][:, s0:s0 + W], ps2[i][:],
                            bk_t[:, hp:hp + 1])

            def pass_Q(w, g, on_act):
                s0 = w * W
                ps2 = [psP.tile([128, 512], FP32, tag="pp",
                                name=f"qp{w}_{g}_{i}") for i in range(2)]
                for dc in range(NDC):
                    xt = xt_slice(dc, s0, W)
                    st, sp = (dc == 0), (dc == NDC - 1)
                    for i in range(2):
                        hp = 2 * g + i
                        nc.tensor.matmul(
                            ps2[i][:],
                            wq_tiles[dc][:, hp * 128:(hp + 1) * 128],
                            xt, start=st, stop=sp)
                for i in range(2):
                    hp = 2 * g + i
                    if on_act:
                        nc.scalar.activation(
                            qt_all[w][hp][:], ps2[i][:],
                            mybir.ActivationFunctionType.Identity,
                            bias=bq_t[:, hp:hp + 1], scale=1.0)
                    else:
                        nc.vector.tensor_scalar_add(
                            qt_all[w][hp][:], ps2[i][:],
                            bq_t[:, hp:hp + 1])

            def pass_V(w, g, on_act):
                s0 = w * W + g * 256
                ps2 = [psP.tile([128, 512], FP32, tag="pp",
                                name=f"vp{w}_{g}_{i}") for i in range(2)]
                for dc in range(NDC):
                    xt = xt_slice(dc, s0, 256)
                    st, sp = (dc == 0), (dc == NDC - 1)
                    for i in range(2):
                        nc.tensor.matmul(
                            ps2[i][:],
                            xt[:, i * 128:i * 128 + 128],
                            wv_tiles[dc][:], start=st, stop=sp)
                for i in range(2):
                    kc = w * 4 + g * 2 + i
                    dst = v_tiles[kc][:].rearrange(
                        "p (h e) -> p h e", e=128)[:, :, 0:64]
                    src = ps2[i][:].rearrange(
                        "p (h e) -> p h e", e=64)
                    if on_act:
                        nc.scalar.copy(dst, src)
                    else:
                        nc.vector.tensor_copy(dst, src)

            def passes_for(w):
                return [lambda a, g=g: pass_K(w, g, a) for g in range(2)] + \
                       [lambda a, g=g: pass_V(w, g, a) for g in range(2)] + \
                       [lambda a, g=g: pass_Q(w, g, a) for g in range(2)]

            # pre-attention: just K/Q for head-pairs 0-1 of window 0
            pass_K(0, 0, True)
            pass_Q(0, 0, True)

            # ---- attention ----
            ao_by_w = {}

            def emit_S(w, hp, kcs):
                qt = qt_all[w][hp]
                ets = {}
                for kc in kcs:
                    j = kc - 4 * w
                    lo = max(j, 0) * 128
                    s2 = psB.tile([128, 1024], FP32, tag="pb", name="s2")
                    et = exp_pool.tile([128, 1024], BF16, tag="et")
                    for par in range(2):
                        nc.tensor.matmul(
                            s2[:, par * 512 + lo:par * 512 + 512],
                            kt_tiles[hp][par * 64:(par + 1) * 64,
                                         kc * KCH:(kc + 1) * KCH],
                            qt[par * 64:(par + 1) * 64, lo:W],
                            start=True, stop=True)
                    if j >= 0:
                        sv = s2[:].rearrange("p (two n) -> p two n",
                                             two=2)[:, :, lo:lo + 128]
                        tv = tri[:].rearrange("p (two n) -> p two n", two=2)
                        nc.vector.tensor_add(sv, sv, tv)
                    if lo == 0:
                        nc.scalar.activation(et[:], s2[:], EXP,
                                             bias=0.0, scale=SCALE)
                    else:
                        sv = s2[:].rearrange("p (two n) -> p two n",
                                             two=2)[:, :, lo:512]
                        ev = et[:].rearrange("p (two n) -> p two n",
                                             two=2)[:, :, lo:512]
                        nc.scalar.activation(ev, sv, EXP,
                                             bias=0.0, scale=SCALE)
                    ets[kc] = et
                return ets

            def emit_V(w, hp, o2, kcs, ets):
                nkc = 4 * (w + 1)
                for kc in kcs:
                    j = kc - 4 * w
                    lo = max(j, 0) * 128
                    for par in range(2):
                        h = 2 * hp + par
                        nc.tensor.matmul(
                            o2[0:128, par * 512 + lo:par * 512 + 512],
                            v_tiles[kc][:, h * 128:(h + 1) * 128],
                            ets[kc][:, par * 512 + lo:par * 512 + 512],
                            start=(kc == 0), stop=(kc == nkc - 1))

            def emit_norm(w, hp, o2):
                zc = zz_pool.tile([64, 1024], FP32, tag="zc")
                nc.vector.tensor_copy(zc[:], o2[64:128, :])
                zi = zz_pool.tile([64, 1024], FP32, tag="zi")
                for par in range(2):
                    nc.vector.reciprocal_approx_fast(
                        out=zi[:, par * 512:par * 512 + 512],
                        in_=zc[:, par * 512:par * 512 + 512])
                    nc.vector.tensor_mul(
                        ao_by_w[w][hp][par * 64:(par + 1) * 64, :],
                        o2[0:65, par * 512:par * 512 + 512][0:64, :],
                        zi[:, par * 512:par * 512 + 512])

            def emit_O_piece(w, qs):
                ao_tiles = ao_by_w[w]
                op2 = psB.tile([128, 1024], FP32, tag="pb", name="op2")
                for dcol in range(2):
                    for hc in range(4):
                        nc.tensor.matmul(
                            op2[:, dcol * 512:dcol * 512 + 512],
                            ao_tiles[hc][:, qs * 128:(qs + 1) * 128],
                            wo_tiles[(hc, dcol)][:],
                            start=(hc == 0), stop=(hc == 3))
                ot = out_pool.tile([128, 1024], FP32, tag="ob")
                nc.vector.tensor_copy(ot[:], op2[:])
                nc.sync.dma_start(
                    out=out[w * W + qs * 128:w * W + (qs + 1) * 128, :],
                    in_=ot[:])

            for w in range(NW):
                nkc = 4 * (w + 1)
                ao_by_w[w] = [ao_pool.tile([128, W], BF16, tag="ao",
                                           name=f"ao{w}_{i}")
                              for i in range(4)]
                batch_starts = [(hp, kc0) for hp in range(4)
                                for kc0 in range(0, nkc, 3)]
                nslots = len(batch_starts)
                # items to interleave into this window's batch slots
                items = []
                if w == 0:
                    # remaining window-0 projections (evictions on ACT:
                    # it is mostly idle during the short window 0)
                    items += [lambda g=g: pass_V(0, g, True) for g in range(2)]
                    items += [lambda: pass_K(0, 1, True),
                              lambda: pass_Q(0, 1, True)]
                else:
                    # previous window's output projection, in 4 pieces
                    items += [lambda qs=qs, pw=w - 1: emit_O_piece(pw, qs)
                              for qs in range(4)]
                if w + 1 < NW:
                    items += [lambda p=p: p(False) for p in passes_for(w + 1)]
                slot_items = {}
                for idx, it in enumerate(items):
                    s = 1 + (idx * (nslots - 1)) // len(items)
                    slot_items.setdefault(s, []).append(it)
                pending = []
                done_hp = set()

                def drain_one(cur_hp):
                    p_hp, p_kcs, p_ets, p_o2 = pending.pop(0)
                    emit_V(w, p_hp, p_o2, p_kcs, p_ets)
                    if (not pending or pending[0][0] != p_hp) \
                            and p_hp != cur_hp and p_hp not in done_hp:
                        done_hp.add(p_hp)
                        emit_norm(w, p_hp, p_o2)

                slot = 0
                cur_hp = None
                for hp, kc0 in batch_starts:
                    if cur_hp != hp:
                        cur_hp = hp
                        o2 = psO.tile([128, 1024], FP32, tag="po",
                                      name=f"o2_{w}_{hp}")
                    kcs = list(range(kc0, min(kc0 + 3, nkc)))
                    ets = emit_S(w, hp, kcs)
                    for it in slot_items.get(slot, []):
                        it()
                    if len(pending) >= 2:
                        drain_one(hp)
                    pending.append((hp, kcs, ets, o2))
                    slot += 1
                while pending:
                    drain_one(None)
            for qs in range(4):
                emit_O_piece(NW - 1, qs)

    nc.compile()
    return nc


def _get_program():
    if "nc" not in _CACHE:
        _CACHE["nc"] = _build_program()
    return _CACHE["nc"]


def _install_ntff_hook():
    """The agent image's antenv lacks axon_hooks; shim it and register the
    ctypes NTFF profiling hook so trace=True yields exec_time_ns."""
    import sys, types
    if "antenv.axon_hooks" in sys.modules:
        return
    try:
        import antenv
        mod = types.ModuleType("antenv.axon_hooks")
        _h = [None]
        mod.set_axon_ntff_profile_hook = lambda h: _h.__setitem__(0, h)
        mod.get_axon_ntff_profile_hook = lambda: _h[0]
        sys.modules["antenv.axon_hooks"] = mod
        antenv.axon_hooks = mod
        from trn_agent_boot.trn_boot import _ntff_profile_via_ctypes
        mod.set_axon_ntff_profile_hook(
            _ntff_profile_via_ctypes("/opt/axon/libaxon_pjrt.so"))
    except Exception as e:  # degrade: run without tracing
        print(f"NTFF hook install failed ({e}); tracing disabled")


def _run(inputs, trace=False):
    x = np.asarray(inputs["x"], dtype=np.float32)
    Wq = np.asarray(inputs["Wq"], dtype=np.float32)
    Wk = np.asarray(inputs["Wk"], dtype=np.float32)
    Wv = np.asarray(inputs["Wv"], dtype=np.float32)
    Wo = np.asarray(inputs["Wo"], dtype=np.float32)
    bq = np.asarray(inputs["bq"], dtype=np.float32)
    bk = np.asarray(inputs["bk"], dtype=np.float32)
    bv = np.asarray(inputs["bv"], dtype=np.float32)
    bo = np.asarray(inputs["bo"], dtype=np.float32)

    if trace:
        _install_ntff_hook()
    nc = _get_program()
    in_maps = []
    for c in range(NCORES):
        b, g = divmod(c, 2)
        sl = slice(g * GD, (g + 1) * GD)
        in_maps.append({
            "xT": np.ascontiguousarray(x[b].T).astype(BF16NP),
            "wq": np.ascontiguousarray(Wq[:, sl]).astype(BF16NP),
            "wk": np.ascontiguousarray(Wk[:, sl]).astype(BF16NP),
            "wv": np.ascontiguousarray(Wv[:, sl]).astype(BF16NP),
            "wo": np.ascontiguousarray(Wo[sl, :]).astype(BF16NP),
            "bq2": np.ascontiguousarray(bq[sl].reshape(4, 128).T),
            "bk2": np.ascontiguousarray(bk[sl].reshape(4, 128).T),
        })
    res = run_bass_kernel_spmd(nc, in_maps, list(range(NCORES)), trace=trace)
    outp = np.empty((B, S, D), dtype=np.float32)
    # bv correction: attention rows sum to 1, so x @ Wv + bv contributes
    # attn@V + bv per row; bv flows through Wo as a constant row vector.
    corr = (bv @ Wo + bo).astype(np.float32)
    for b in range(B):
        outp[b] = res.results[2 * b]["out"] + res.results[2 * b + 1]["out"] + corr
    return outp, res


def kernel(**inputs):
    outp, _ = _run(inputs, trace=False)
    return outp


def kernel_traced(**inputs):
    outp, res = _run(inputs, trace=True)
    return outp, res
